# revision 1
# baseline (speedup 1.0000x reference)
"""GCN layer (gather + segment_sum + scale) on 8 Trainium2 NeuronCores.

Strategy (1D destination-node parallel):
  - Host (integer/index work only): shard edges by dst block of 12500 nodes
    (core i owns dst nodes [12500*i, 12500*(i+1))). Per core, sort owned
    nodes by in-degree, pack them into 98 groups of 128 nodes. Group g gets
    k_g "slot tiles": slot (p, c) holds the j-th in-edge of node rank
    g*128+p, padded with a pointer to an all-zero table row when j >= deg.
    Groups with equal k are batched so device-side adds are wide.
  - Device (all FP math): table rows are [node_f || out_d] (33 f32).
    Chunked indirect-DMA gather (SWDGE) of 128-column slot blocks ->
    DVE scale-by-out_d (pack to 32 wide) -> DVE wide accumulate into an
    SBUF accumulator [128, 98, 32] -> multiply by in_dg -> DMA out.
  - Host: inverse-permute rows back to original node order.
"""

import sys

import numpy as np

for _p in ("/opt/trn_rl_repo", "/root/.axon_site/_ro/trn_rl_repo"):
    if _p not in sys.path:
        sys.path.append(_p)

P = 128
D = 32
TW = D + 1  # table row width: 32 features + out_d
CHUNK_COLS = 128
N_CORES = 8

_cache = {}


# ---------------------------------------------------------------- host prep


def _segments(k_g, chunk_cols):
    """List of (chunk, cs_local, w, gs, j): add (or copy when j==0)
    pk[chunk][:, cs:cs+w, :] into acc[:, gs:gs+w, :]."""
    G = len(k_g)
    runs = []
    g0 = 0
    while g0 < G:
        g1 = g0
        while g1 < G and k_g[g1] == k_g[g0]:
            g1 += 1
        runs.append((g0, g1, int(k_g[g0])))
        g0 = g1
    segs = []
    c = 0
    colbase = {}
    # emit largest-k batches first so the schedule tail is a small batch's
    # finalize, not the big one's
    for (g0, g1, k) in reversed(runs):
        b = g1 - g0
        for j in range(k):
            colbase[(g0, j)] = c
            s = c
            while s < c + b:
                e = min(c + b, (s // chunk_cols + 1) * chunk_cols)
                # last=True marks the final accumulation into this group range:
                # finalize (in_dg multiply + output DMA) can fire right after.
                segs.append(
                    (s // chunk_cols, s % chunk_cols, e - s, g0 + (s - c), j, j == k - 1)
                )
                s = e
            c += b
    return segs, colbase, c


def _preprocess(node_f, out_d, in_dg, src, dst):
    n = node_f.shape[0]
    npc = n // N_CORES  # nodes per core
    G = (npc + P - 1) // P
    node_slots = G * P

    table = np.zeros((n + 1, TW), dtype=np.float32)
    table[:n, :D] = node_f
    table[:n, D] = out_d[:, 0]

    core_of = dst // npc
    per_core = []
    deg_all = []
    for i in range(N_CORES):
        m = core_of == i
        e_src = src[m].astype(np.int64)
        e_dstl = (dst[m] - i * npc).astype(np.int64)
        # group edges by local dst
        perm = np.argsort(e_dstl, kind="stable")
        e_src = e_src[perm]
        e_dstl = e_dstl[perm]
        deg = np.bincount(e_dstl, minlength=npc)
        deg_ext = np.zeros(node_slots, dtype=np.int64)
        deg_ext[:npc] = deg
        order = np.argsort(deg_ext, kind="stable")  # rank -> node slot
        per_core.append((e_src, e_dstl, deg, deg_ext, order))
        deg_all.append(deg_ext[order])  # sorted degrees

    # global slot count per group = max over cores of group-max degree
    k_g = np.zeros(G, dtype=np.int64)
    for i in range(N_CORES):
        sd = deg_all[i]
        k_g = np.maximum(k_g, sd.reshape(G, P).max(axis=1))
    # >=1 so every acc region gets its j==0 copy (first-touch init)
    k_g = np.maximum(k_g, 1)

    segs, colbase, C = _segments(k_g, CHUNK_COLS)
    C_pad = C  # no chunk rounding: the last chunk is emitted partial-width

    # colOf lookup table for vectorized idx fill
    kmax = int(k_g.max())
    colOf = np.full((G, kmax), C_pad - 1, dtype=np.int64)  # default -> harmless
    g = 0
    while g < G:
        g1 = g
        while g1 < G and k_g[g1] == k_g[g]:
            g1 += 1
        for j in range(int(k_g[g])):
            colOf[g:g1, j] = colbase[(g, j)] + np.arange(g1 - g)
        g = g1

    idx_arrs = np.full((N_CORES, P, C_pad), n, dtype=np.int32)  # n = zero row
    indg_arrs = np.zeros((N_CORES, P, G, 1), dtype=np.float32)
    orders = []
    for i in range(N_CORES):
        e_src, e_dstl, deg, deg_ext, order = per_core[i]
        rank_of = np.empty(node_slots, dtype=np.int64)
        rank_of[order] = np.arange(node_slots)
        off = np.zeros(npc + 1, dtype=np.int64)
        np.cumsum(deg, out=off[1:])
        j_e = np.arange(len(e_src)) - off[e_dstl]
        r_e = rank_of[e_dstl]
        col_e = colOf[r_e // P, j_e]
        idx_arrs[i, r_e % P, col_e] = e_src
        rr = np.arange(node_slots)
        real = order < npc
        indg_arrs[i, rr[real] % P, rr[real] // P, 0] = in_dg[i * npc + order[real], 0]
        orders.append(order)

    return dict(
        table=table,
        idx=idx_arrs,
        indg=indg_arrs,
        orders=orders,
        segs=segs,
        C_pad=C_pad,
        G=G,
        npc=npc,
        n=n,
        slots_real=int(sum(len(pc[0]) for pc in per_core)),
        slots_total=int(N_CORES * P * C_pad),
    )


# ---------------------------------------------------------------- device


def _build_nc(n_table_rows, C_pad, G, segs):
    import concourse.bass as bass
    import concourse.tile as tile
    from concourse import bacc, mybir

    nc = bacc.Bacc("TRN2", target_bir_lowering=False, debug=False)
    table_d = nc.dram_tensor(
        "table", [n_table_rows, TW], mybir.dt.float32, kind="ExternalInput"
    ).ap()
    idx_d = nc.dram_tensor("idx", [P, C_pad], mybir.dt.int32, kind="ExternalInput").ap()
    indg_d = nc.dram_tensor(
        "indg", [P, G, 1], mybir.dt.float32, kind="ExternalInput"
    ).ap()
    out_dram = nc.dram_tensor(
        "out", [P, G, D], mybir.dt.float32, kind="ExternalOutput"
    ).ap()

    n_chunks = (C_pad + CHUNK_COLS - 1) // CHUNK_COLS
    segs_by_chunk = {}
    for (ch, cs, w, gs, j, last) in segs:
        segs_by_chunk.setdefault(ch, []).append((cs, w, gs, j, last))

    with tile.TileContext(nc) as tc:
        with (
            tc.tile_pool(name="persist", bufs=1) as persist,
            tc.tile_pool(name="idxp", bufs=4) as idx_pool,
            tc.tile_pool(name="msgs", bufs=4) as msgs_pool,
            tc.tile_pool(name="packed", bufs=3) as packed_pool,
        ):
            indg_t = persist.tile([P, G, 1], mybir.dt.float32)
            acc = persist.tile([P, G, D], mybir.dt.float32)
            nc.sync.dma_start(out=indg_t[:], in_=indg_d[:])
            for ch in range(n_chunks):
                cols = min(CHUNK_COLS, C_pad - ch * CHUNK_COLS)
                # just-in-time index load: chunk 0's gathers start after a
                # 64KB transfer instead of the whole index array
                idx_t = idx_pool.tile([P, cols], mybir.dt.int32, tag="ix")
                nc.sync.dma_start(
                    out=idx_t[:],
                    in_=idx_d[:, ch * CHUNK_COLS : ch * CHUNK_COLS + cols],
                )
                m = msgs_pool.tile([P, cols, TW], mybir.dt.float32, tag="m")
                for c in range(cols):
                    # HW indirect DMA semantics: one descriptor per partition,
                    # row = table[idx[p]] -> dest partition line (2D AP only).
                    nc.gpsimd.indirect_dma_start(
                        out=m[:, c, :],
                        out_offset=None,
                        in_=table_d[:],
                        in_offset=bass.IndirectOffsetOnAxis(
                            ap=idx_t[:, c : c + 1], axis=0
                        ),
                    )
                pk = packed_pool.tile([P, cols, D], mybir.dt.float32, tag="pk")
                nc.vector.tensor_tensor(
                    out=pk[:],
                    in0=m[:, :, 0:D],
                    in1=m[:, :, D : D + 1].to_broadcast([P, cols, D]),
                    op=mybir.AluOpType.mult,
                )
                for (cs, w, gs, j, last) in segs_by_chunk.get(ch, []):
                    if j == 0:
                        # first tile of the group: init the acc region
                        nc.vector.tensor_copy(
                            out=acc[:, gs : gs + w, :], in_=pk[:, cs : cs + w, :]
                        )
                    else:
                        nc.vector.tensor_tensor(
                            out=acc[:, gs : gs + w, :],
                            in0=pk[:, cs : cs + w, :],
                            in1=acc[:, gs : gs + w, :],
                            op=mybir.AluOpType.add,
                        )
                    if last:
                        # final accumulation for groups [gs, gs+w): scale by
                        # in_dg and ship out now, overlapping later chunks
                        nc.vector.tensor_tensor(
                            out=acc[:, gs : gs + w, :],
                            in0=acc[:, gs : gs + w, :],
                            in1=indg_t[:, gs : gs + w, :].to_broadcast([P, w, D]),
                            op=mybir.AluOpType.mult,
                        )
                        nc.sync.dma_start(
                            out=out_dram[:, gs : gs + w, :],
                            in_=acc[:, gs : gs + w, :],
                        )
    nc.compile()
    return nc


# ---------------------------------------------------------------- entry


last_run_info = {}


def kernel(node_f, out_d, in_dg, src, dst, *, _trace=False):
    node_f = np.asarray(node_f, dtype=np.float32)
    out_d = np.asarray(out_d, dtype=np.float32)
    in_dg = np.asarray(in_dg, dtype=np.float32)
    src = np.asarray(src)
    dst = np.asarray(dst)

    pp = _preprocess(node_f, out_d, in_dg, src, dst)

    key = (pp["n"], pp["C_pad"], pp["G"], len(pp["segs"]))
    if key not in _cache:
        _cache.clear()
        _cache[key] = _build_nc(pp["n"] + 1, pp["C_pad"], pp["G"], pp["segs"])
    nc = _cache[key]

    from concourse.bass_utils import run_bass_kernel_spmd

    in_maps = [
        {"table": pp["table"], "idx": pp["idx"][i], "indg": pp["indg"][i]}
        for i in range(N_CORES)
    ]
    # Sacrificial device touch: after an earlier crashed session the first
    # device interaction can report NRT_EXEC_UNIT_UNRECOVERABLE once and
    # then recover; absorb that here instead of failing the real run.
    try:
        import jax
        import jax.numpy as jnp

        jnp.zeros((2,)).block_until_ready()
    except Exception:
        pass

    trace_kwargs = (
        dict(trace=True, trace_cores=list(range(N_CORES))) if _trace else {}
    )
    res = None
    last_exc = None
    for attempt in range(3):
        try:
            res = run_bass_kernel_spmd(
                nc, in_maps, core_ids=list(range(N_CORES)), **trace_kwargs
            )
            break
        except ModuleNotFoundError:
            # NTFF profiling hook unavailable in this environment
            trace_kwargs = {}
        except Exception as e:  # noqa: BLE001
            last_exc = e
            import time as _time

            _time.sleep(2.0)
    if res is None:
        res = run_bass_kernel_spmd(nc, in_maps, core_ids=list(range(N_CORES)))
    last_run_info["exec_time_ns"] = res.exec_time_ns
    last_run_info["mean_exec_time_ns"] = res.mean_exec_time_ns
    last_run_info["trace"] = res.instructions_and_trace
    last_run_info["pp_stats"] = {
        k: pp[k] for k in ("C_pad", "G", "slots_real", "slots_total")
    }

    n, npc, G = pp["n"], pp["npc"], pp["G"]
    out = np.empty((n, D), dtype=np.float32)
    rr = np.arange(G * P)
    for i in range(N_CORES):
        o = res.results[i]["out"]  # [P, G, D]
        order = pp["orders"][i]
        real = order < npc
        out[i * npc + order[real]] = o[rr[real] % P, rr[real] // P]
    return out



# revision 5
# speedup vs baseline: 6.3667x; 6.3667x over previous
"""GCN layer (gather + segment_sum + scale) on 8 Trainium2 NeuronCores.

Strategy (1D destination-node parallel, dma_gather based):
  - Host (integer/index/layout work only): shard edges by dst block of 12500
    nodes. Per core, pack nodes into 98 groups of 128 ranks (greedy packing
    that minimizes per-chunk degree maxima). Edges are keyed by src chunk
    (4 chunks of 25024 table rows, the int16 addressing limit of
    dma_gather). Slot (p, c): column c belongs to (chunk, layer j, group g);
    row p = node rank % 128. Padding slots point at a per-chunk zero row.
  - Device (all FP math): build a prescaled bf16 table h = node_f * out_d
    in DRAM with 256B row stride (dma_gather ISA requirement), then one
    dma_gather per 64 columns (8192 indices, 64B payloads). The Tensor
    engine accumulates slot columns into PSUM (fp32) via identity matmuls;
    finalize multiplies by in_dg on DVE and DMAs out.
  - Host: inverse-permute rows back to original node order.
"""

import sys

import numpy as np

for _p in ("/opt/trn_rl_repo", "/root/.axon_site/_ro/trn_rl_repo"):
    if _p not in sys.path:
        sys.path.append(_p)

P = 128
D = 32
N_CORES = 8
N = 100000
NPC = N // N_CORES            # 12500 dst nodes per core
G = (NPC + P - 1) // P        # 98 groups
RANKS = G * P                 # 12544
CHROWS = 25024                # real table rows per chunk
CHSTRIDE = 25056              # chunk stride in table rows (zero row at 25024)
NCH = 4
TROWS = NCH * CHSTRIDE        # 100224 table rows
TW = 128                      # table row width in bf16 elems (256B stride)
TILE_COLS = 64                # columns per dma_gather (8192 idxs)
BANK = 16                     # psum groups per bank
NB = (G + BANK - 1) // BANK   # 7 banks

_cache = {}


# ---------------------------------------------------------------- host prep


def _group_nodes(deg_ch):
    """Greedy pack nodes into groups of P minimizing sum of per-chunk maxima.

    deg_ch: [NPC, NCH] per-node per-chunk degree. Returns rank_to_node
    [RANKS] (node id or -1) and k (group, chunk) maxima [G, NCH].
    """
    order = np.argsort(-deg_ch.max(axis=1), kind="stable")
    gmax = np.zeros((G, NCH), np.int64)
    gsum = np.zeros(G, np.int64)
    gcnt = np.zeros(G, np.int64)
    assign = np.empty(NPC, np.int64)
    BIG = 1 << 40
    for v in order:
        d = deg_ch[v]
        inc = np.maximum(gmax, d).sum(axis=1) - gsum
        inc[gcnt >= P] = BIG
        g = int(inc.argmin())
        assign[v] = g
        np.maximum(gmax[g], d, out=gmax[g])
        gsum[g] = gmax[g].sum()
        gcnt[g] += 1
    rank_to_node = np.full(RANKS, -1, np.int64)
    pos = np.zeros(G, np.int64)
    for v in range(NPC):
        g = assign[v]
        rank_to_node[g * P + pos[g]] = v
        pos[g] += 1
    return rank_to_node, gmax


def _preprocess(node_f, out_d, in_dg, src, dst):
    core_of = dst // NPC
    chunk_of_all = src // CHROWS

    per_core = []
    k_sorted_all = []
    for i in range(N_CORES):
        m = core_of == i
        e_src = src[m].astype(np.int64)
        e_dstl = (dst[m] - i * NPC).astype(np.int64)
        e_ch = chunk_of_all[m].astype(np.int64)
        deg_ch = np.zeros((NPC, NCH), np.int64)
        for c in range(NCH):
            deg_ch[:, c] = np.bincount(e_dstl[e_ch == c], minlength=NPC)
        rank_to_node, k = _group_nodes(deg_ch)
        # sort groups by total k desc so per-core profiles align across cores
        gorder = np.argsort(-k.sum(axis=1), kind="stable")
        k_sorted = k[gorder]
        rank_to_node = rank_to_node.reshape(G, P)[gorder].reshape(RANKS)
        per_core.append((e_src, e_dstl, e_ch, rank_to_node))
        k_sorted_all.append(k_sorted)

    k_shared = np.maximum.reduce(k_sorted_all)      # [G, NCH]
    k_shared[:, 0] = np.maximum(k_shared[:, 0], 1)  # chunk-0 layer-0 = full init

    # global column order: chunk-major, then layer j, then group g ascending
    cols = []          # (ch, j, g)
    colpos = {}        # ch -> [kmax_ch, G] int32 col position (chunk-local)
    C_ch = []
    C_ch_pad = []
    for ch in range(NCH):
        kmax = int(k_shared[:, ch].max())
        cp = np.full((max(kmax, 1), G), -1, np.int64)
        c = 0
        for j in range(kmax):
            for g in range(G):
                if k_shared[g, ch] > j:
                    cp[j, g] = c
                    cols.append((ch, j, g))
                    c += 1
        pad = (-c) % TILE_COLS
        colpos[ch] = cp
        C_ch.append(c)
        C_ch_pad.append(c + pad)
        cols.extend([(ch, -1, -1)] * pad)
    C_tot = sum(C_ch_pad)

    chunk_col_base = np.concatenate([[0], np.cumsum(C_ch_pad)])[:NCH]

    # idx arrays: wrapped 16-partition layout per 64-col (8192-idx) tile,
    # replicated across the 8 Q7 cores (partitions 16..127)
    idx_all = np.empty((N_CORES, 128, C_tot * 8), dtype=np.int16)
    indg_all = np.zeros((N_CORES, P, G, 1), dtype=np.float32)
    for i in range(N_CORES):
        e_src, e_dstl, e_ch, rank_to_node = per_core[i]
        rank_of = np.full(NPC, -1, np.int64)
        real = rank_to_node >= 0
        rank_of[rank_to_node[real]] = np.nonzero(real)[0]
        r_e = rank_of[e_dstl]
        # j index within (rank, chunk): stable sort then cumcount
        key = r_e * NCH + e_ch
        perm = np.argsort(key, kind="stable")
        ks = key[perm]
        starts = np.r_[0, np.nonzero(np.diff(ks))[0] + 1]
        runlen = np.diff(np.r_[starts, len(ks)])
        j_sorted = np.arange(len(ks)) - np.repeat(starts, runlen)
        j_e = np.empty(len(ks), np.int64)
        j_e[perm] = j_sorted
        # column of each edge (chunk-local), then linear slot index
        col_l = np.empty(len(ks), np.int64)
        for ch in range(NCH):
            mm = e_ch == ch
            col_l[mm] = colpos[ch][j_e[mm], r_e[mm] // P]
        assert (col_l >= 0).all()
        slots = np.full(C_tot * 128, CHROWS, dtype=np.int16)  # default zero row
        lin = (chunk_col_base[e_ch] + col_l) * 128 + (r_e % P)
        slots[lin] = (e_src - e_ch * CHROWS).astype(np.int16)
        # wrap per 8192-idx tile: tile t covers slots [t*8192, (t+1)*8192)
        sl = slots.reshape(-1, 8192)            # [n_tiles, 8192]
        w = sl.reshape(-1, 512, 16).transpose(0, 2, 1)  # [n_tiles, 16, 512]
        wr = idx_all[i]
        w16 = np.ascontiguousarray(w.transpose(1, 0, 2)).reshape(16, -1)
        for r in range(8):
            wr[16 * r:16 * (r + 1)] = w16
        rr = np.arange(RANKS)
        indg_all[i, rr[real] % P, rr[real] // P, 0] = in_dg[
            i * NPC + rank_to_node[real], 0
        ]

    # matmul segments: runs of consecutive cols with same (ch, j), consecutive
    # g, same 64-col tile, same psum bank
    segs = []  # (tile, col_start_in_tile, width, g_start, ch, j)
    prev = None
    for pos, (ch, j, g) in enumerate(cols):
        if j < 0:
            if prev is not None:
                segs.append(tuple(prev))
                prev = None
            continue
        t, cs = divmod(pos, TILE_COLS)
        if (
            prev is not None
            and prev[4] == ch
            and prev[5] == j
            and g == prev[3] + prev[2]
            and t == prev[0]
            and g // BANK == prev[3] // BANK
        ):
            prev[2] += 1
        else:
            if prev is not None:
                segs.append(tuple(prev))
            prev = [t, cs, 1, g, ch, j]
    if prev is not None:
        segs.append(tuple(prev))

    nf_pad = np.zeros((NCH * CHROWS, D), dtype=np.float32)
    nf_pad[:N] = node_f
    od_pad = np.zeros((NCH * CHROWS, 1), dtype=np.float32)
    od_pad[:N] = out_d

    return dict(
        nf=nf_pad,
        od=od_pad,
        idx=idx_all,
        indg=indg_all,
        rank_to_node=[pc[3] for pc in per_core],
        segs=segs,
        C_tot=C_tot,
        n_tiles=C_tot // TILE_COLS,
        slots_real=int(len(src)),
        slots_total=int(C_tot * 128 * N_CORES),
    )


# ---------------------------------------------------------------- device


def _build_nc(C_tot, n_tiles, segs):
    import concourse.bass as bass
    import concourse.tile as tile
    from concourse import bacc, mybir

    nc = bacc.Bacc(
        "TRN2", target_bir_lowering=False, debug=False,
        dynamic_dma_scratch_size=32768,
    )
    nf_d = nc.dram_tensor(
        "nf", [NCH * CHROWS, D], mybir.dt.float32, kind="ExternalInput"
    ).ap()
    od_d = nc.dram_tensor(
        "od", [NCH * CHROWS, 1], mybir.dt.float32, kind="ExternalInput"
    ).ap()
    idx_d = nc.dram_tensor(
        "idx", [128, C_tot * 8], mybir.dt.int16, kind="ExternalInput"
    ).ap()
    indg_d = nc.dram_tensor(
        "indg", [P, G, 1], mybir.dt.float32, kind="ExternalInput"
    ).ap()
    out_dram = nc.dram_tensor(
        "out", [P, G, D], mybir.dt.float32, kind="ExternalOutput"
    ).ap()

    def raw_dma_gather(out_ap, in_ap, idxs_ap, num_idxs, elem_size, elem_step):
        eng = nc.gpsimd
        stride_bytes = elem_step * mybir.dt.size(in_ap.dtype)
        _in_ap = eng.lower_ap_dma(in_ap, for_custom_bir_dma=True)
        return eng.add_instruction(
            mybir.InstDMAGatherAnt(
                name=eng.bass.get_next_instruction_name(),
                ins=[*_in_ap, eng.lower_ap(idxs_ap),
                     eng.lower_val_access(eng.to_reg(num_idxs))],
                outs=[eng.lower_ap(out_ap)],
                transpose=False, num_idxs=num_idxs, elem_size=elem_size,
                stride_bytes_256=stride_bytes // 256, gen_mode=0,
                single_packet=False, queue_num=0, sbuf_tokens_per_rank=0,
                sbuf_free_dim_per_rank=0, sbuf_free_dim_pad_per_rank=0,
                sbuf_byte_offset=0,
            )
        )

    # per-bank first/last matmul seg index for start/stop/finalize
    last_of_bank = {}
    for si, (t, cs, w, gs, ch, j) in enumerate(segs):
        last_of_bank[gs // BANK] = si

    with tile.TileContext(nc) as tc:
        with (
            tc.tile_pool(name="persist", bufs=1) as persist,
            tc.tile_pool(name="build", bufs=1) as build_pool,
            tc.tile_pool(name="tabp", bufs=1, space="DRAM") as tab_pool,
            tc.tile_pool(name="msgs", bufs=4) as msgs_pool,
            tc.psum_pool(name="psum", bufs=1) as psum_pool,
        ):
            table_t = tab_pool.tile([TROWS, TW], mybir.dt.bfloat16)

            # ---- table build: h = bf16(node_f) * bf16(out_d)
            h_sb = build_pool.tile([128, 782, D], mybir.dt.bfloat16)
            od_sb = build_pool.tile([128, 782, 1], mybir.dt.bfloat16)
            nc.gpsimd.dma_start(out=h_sb[:], in_=nf_d[:])
            nc.gpsimd.dma_start(out=od_sb[:], in_=od_d[:])
            nc.vector.tensor_tensor(
                out=h_sb[:], in0=h_sb[:],
                in1=od_sb[:].to_broadcast([128, 782, D]),
                op=mybir.AluOpType.mult,
            )
            zrow = build_pool.tile([1, D], mybir.dt.bfloat16)
            nc.vector.memset(zrow[:], 0)
            for ch in range(NCH):
                nc.sync.dma_start(
                    out=table_t[CHSTRIDE * ch:CHSTRIDE * ch + CHROWS, 0:D],
                    in_=h_sb[32 * ch:32 * (ch + 1), :, :],
                )
                nc.sync.dma_start(
                    out=table_t[CHSTRIDE * ch + CHROWS:CHSTRIDE * ch + CHROWS + 1, 0:D],
                    in_=zrow[:],
                )

            # ---- small persistent tiles
            idx_t = persist.tile([128, C_tot * 8], mybir.dt.int16)
            nc.sync.dma_start(out=idx_t[:], in_=idx_d[:])
            indg_t = persist.tile([P, G, 1], mybir.dt.float32)
            nc.sync.dma_start(out=indg_t[:], in_=indg_d[:])
            it = persist.tile([P, P], mybir.dt.int32)
            nc.gpsimd.iota(it[:], pattern=[[1, P]], base=0, channel_multiplier=-1)
            ident = persist.tile([P, P], mybir.dt.bfloat16)
            nc.vector.tensor_scalar(
                out=ident[:], in0=it[:], scalar1=0, scalar2=None,
                op0=mybir.AluOpType.is_equal,
            )

            pt = psum_pool.tile([P, NB * BANK, D], mybir.dt.float32)
            res = persist.tile([P, G, D], mybir.dt.float32)

            # ---- gather + accumulate
            segs_by_tile = {}
            for si, s in enumerate(segs):
                segs_by_tile.setdefault(s[0], []).append((si, s))

            # tiles are chunk-aligned; map tile -> chunk from segs (pad-only
            # tiles inherit the previous tile's chunk)
            tmap = {}
            for (t, cs, w, gs, ch, j) in segs:
                tmap[t] = ch
            tile_chunk = []
            prev_ch = 0
            for t in range(n_tiles):
                ch = tmap.get(t, prev_ch)
                prev_ch = ch
                tile_chunk.append(ch)

            for t in range(n_tiles):
                ch = tile_chunk[t]
                m = msgs_pool.tile([P, TILE_COLS, D], mybir.dt.bfloat16, tag="m")
                raw_dma_gather(
                    m[:],
                    table_t[CHSTRIDE * ch:CHSTRIDE * (ch + 1), 0:D],
                    idx_t[:, t * 512:(t + 1) * 512],
                    TILE_COLS * 128, D, TW,
                )
                for si, (tt, cs, w, gs, sch, j) in segs_by_tile.get(t, []):
                    nc.tensor.matmul(
                        pt[:, gs:gs + w, :],
                        ident[:],
                        m[:, cs:cs + w, :],
                        start=(sch == 0 and j == 0),
                        stop=(last_of_bank[gs // BANK] == si),
                    )
                    b = gs // BANK
                    if last_of_bank[b] == si:
                        g0 = b * BANK
                        wb = min(BANK, G - g0)
                        nc.vector.tensor_tensor(
                            out=res[:, g0:g0 + wb, :],
                            in0=pt[:, g0:g0 + wb, :],
                            in1=indg_t[:, g0:g0 + wb, :].to_broadcast([P, wb, D]),
                            op=mybir.AluOpType.mult,
                        )
                        nc.sync.dma_start(
                            out=out_dram[:, g0:g0 + wb, :],
                            in_=res[:, g0:g0 + wb, :],
                        )
    nc.compile()
    return nc


# ---------------------------------------------------------------- entry


last_run_info = {}


def kernel(node_f, out_d, in_dg, src, dst, *, _trace=False):
    node_f = np.asarray(node_f, dtype=np.float32)
    out_d = np.asarray(out_d, dtype=np.float32)
    in_dg = np.asarray(in_dg, dtype=np.float32)
    src = np.asarray(src)
    dst = np.asarray(dst)

    pp = _preprocess(node_f, out_d, in_dg, src, dst)

    key = (pp["C_tot"], len(pp["segs"]))
    if key not in _cache:
        _cache.clear()
        _cache[key] = _build_nc(pp["C_tot"], pp["n_tiles"], pp["segs"])
    nc = _cache[key]

    from concourse.bass_utils import run_bass_kernel_spmd

    in_maps = [
        {"nf": pp["nf"], "od": pp["od"], "idx": pp["idx"][i], "indg": pp["indg"][i]}
        for i in range(N_CORES)
    ]
    # Sacrificial device touch: after an earlier crashed session the first
    # device interaction can report NRT_EXEC_UNIT_UNRECOVERABLE once and
    # then recover; absorb that here instead of failing the real run.
    try:
        import jax
        import jax.numpy as jnp

        jnp.zeros((2,)).block_until_ready()
    except Exception:
        pass

    trace_kwargs = (
        dict(trace=True, trace_cores=list(range(N_CORES))) if _trace else {}
    )
    res = None
    for attempt in range(3):
        try:
            res = run_bass_kernel_spmd(
                nc, in_maps, core_ids=list(range(N_CORES)), **trace_kwargs
            )
            break
        except ModuleNotFoundError:
            trace_kwargs = {}
        except Exception:  # noqa: BLE001
            import time as _time

            _time.sleep(2.0)
    if res is None:
        res = run_bass_kernel_spmd(nc, in_maps, core_ids=list(range(N_CORES)))
    last_run_info["exec_time_ns"] = res.exec_time_ns
    last_run_info["mean_exec_time_ns"] = res.mean_exec_time_ns
    last_run_info["trace"] = res.instructions_and_trace
    last_run_info["pp_stats"] = {
        k: pp[k] for k in ("C_tot", "slots_real", "slots_total")
    }

    out = np.empty((N, D), dtype=np.float32)
    rr = np.arange(RANKS)
    for i in range(N_CORES):
        o = res.results[i]["out"]  # [P, G, D]
        r2n = pp["rank_to_node"][i]
        real = r2n >= 0
        out[i * NPC + r2n[real]] = o[rr[real] % P, rr[real] // P]
    return out


# revision 6
# speedup vs baseline: 6.9800x; 1.0963x over previous
"""GCN layer (gather + segment_sum + scale) on 8 Trainium2 NeuronCores.

Strategy (1D destination-node parallel, dma_gather based):
  - Host (integer/index/layout work only): shard edges by dst block of 12500
    nodes. Sources are 4-colored (greedy, balancing each destination's
    in-edges across colors) because dma_gather indices are int16: each color
    class gets its own table of <= 25088 rows. Per core, nodes are packed
    into 98 groups of 128 ranks by a greedy that minimizes per-color degree
    maxima. Slot (p, c): column c belongs to (class, layer j, group g);
    row p = node rank % 128. Padding slots point at a zeroed pad row.
  - Device (all FP math): build prescaled bf16 class tables h = node_f*out_d
    in DRAM with 256B row stride (dma_gather ISA requirement), then one
    dma_gather per <=64 columns (<=8192 indices, 64B payloads). The Tensor
    engine accumulates slot columns into PSUM (fp32) via identity matmuls;
    finalize multiplies by in_dg on DVE and DMAs out.
  - Host: inverse-permute rows back to original node order.
"""

import sys

import numpy as np

for _p in ("/opt/trn_rl_repo", "/root/.axon_site/_ro/trn_rl_repo"):
    if _p not in sys.path:
        sys.path.append(_p)

P = 128
D = 32
N_CORES = 8
N = 100000
NPC = N // N_CORES            # 12500 dst nodes per core
G = (NPC + P - 1) // P        # 98 groups
RANKS = G * P                 # 12544
Q = 4                         # color classes (int16 addressing limit)
R = 25088                     # class table rows (= 32*784)
CLASS_CAP = R - 64            # max real nodes per class
TW = 128                      # table row width in bf16 elems (256B stride)
TILE_COLS = 64                # max columns per dma_gather (8192 idxs)
BANK = 16                     # psum groups per bank
NB = (G + BANK - 1) // BANK   # 7 banks

_cache = {}


# ---------------------------------------------------------------- host prep


def _color_sources(src, dst):
    """4-color source nodes, balancing each dst's in-edges across colors.

    Returns color [N] int8 and pos [N] (position within class).
    """
    deg_tot = np.bincount(dst, minlength=N)
    cap = np.ceil(deg_tot / Q).astype(np.int32)
    es = np.argsort(src, kind="stable")
    d_sorted = dst[es]
    starts = np.searchsorted(src[es], np.arange(N + 1))
    out_deg = np.diff(starts)
    order = np.argsort(-out_deg, kind="stable")
    deg_q = np.zeros((N, Q), np.int32)
    class_sz = np.zeros(Q, np.int64)
    color = np.empty(N, np.int8)
    for u in order:
        vs = d_sorted[starts[u]:starts[u + 1]]
        if len(vs) == 0:
            q = int(class_sz.argmin())
        else:
            dq = deg_q[vs]
            over = np.maximum(dq + 1 - cap[vs][:, None], 0).sum(axis=0)
            over = over.astype(np.int64) * (1 << 20) + class_sz
            over[class_sz >= CLASS_CAP] = 1 << 60
            q = int(over.argmin())
            deg_q[vs, q] += 1
        color[u] = q
        class_sz[q] += 1
    pos = np.empty(N, np.int64)
    for q in range(Q):
        mq = np.nonzero(color == q)[0]
        pos[mq] = np.arange(len(mq))
    return color, pos, class_sz


def _group_nodes(deg_ch):
    """Greedy pack nodes into groups of P minimizing sum of per-class maxima."""
    order = np.argsort(-deg_ch.max(axis=1), kind="stable")
    gmax = np.zeros((G, Q), np.int64)
    gsum = np.zeros(G, np.int64)
    gcnt = np.zeros(G, np.int64)
    assign = np.empty(NPC, np.int64)
    BIG = 1 << 40
    for v in order:
        d = deg_ch[v]
        inc = np.maximum(gmax, d).sum(axis=1) - gsum
        inc[gcnt >= P] = BIG
        g = int(inc.argmin())
        assign[v] = g
        np.maximum(gmax[g], d, out=gmax[g])
        gsum[g] = gmax[g].sum()
        gcnt[g] += 1
    rank_to_node = np.full(RANKS, -1, np.int64)
    posg = np.zeros(G, np.int64)
    for v in range(NPC):
        g = assign[v]
        rank_to_node[g * P + posg[g]] = v
        posg[g] += 1
    return rank_to_node, gmax


def _preprocess(node_f, out_d, in_dg, src, dst):
    src = src.astype(np.int64)
    dst = dst.astype(np.int64)
    color, cpos, class_sz = _color_sources(src, dst)
    e_col = color[src].astype(np.int64)

    core_of = dst // NPC
    per_core = []
    k_sorted_all = []
    for i in range(N_CORES):
        m = core_of == i
        e_src = src[m]
        e_dstl = dst[m] - i * NPC
        e_ch = e_col[m]
        deg_ch = np.zeros((NPC, Q), np.int64)
        for c in range(Q):
            deg_ch[:, c] = np.bincount(e_dstl[e_ch == c], minlength=NPC)
        rank_to_node, k = _group_nodes(deg_ch)
        gorder = np.argsort(-k.sum(axis=1), kind="stable")
        k_sorted = k[gorder]
        rank_to_node = rank_to_node.reshape(G, P)[gorder].reshape(RANKS)
        per_core.append((e_src, e_dstl, e_ch, rank_to_node))
        k_sorted_all.append(k_sorted)

    k_shared = np.maximum.reduce(k_sorted_all)      # [G, Q]
    k_shared[:, 0] = np.maximum(k_shared[:, 0], 1)  # class-0 layer-0 full init

    # global column order: class-major, then layer j, then group g ascending
    cols = []          # (q, j, g); (q, -1, -1) for pad
    colpos = {}        # q -> [kmax_q, G] chunk-local col position
    C_ch, C_ch_pad = [], []
    for ch in range(Q):
        kmax = int(k_shared[:, ch].max())
        cp = np.full((max(kmax, 1), G), -1, np.int64)
        c = 0
        for j in range(kmax):
            for g in range(G):
                if k_shared[g, ch] > j:
                    cp[j, g] = c
                    cols.append((ch, j, g))
                    c += 1
        pad = (-c) % TILE_COLS
        colpos[ch] = cp
        C_ch.append(c)
        C_ch_pad.append(c + pad)
        cols.extend([(ch, -1, -1)] * pad)
    C_tot = sum(C_ch_pad)
    chunk_col_base = np.concatenate([[0], np.cumsum(C_ch_pad)])[:Q]

    # per-tile: class + number of real columns
    n_tiles = C_tot // TILE_COLS
    tile_ch = np.empty(n_tiles, np.int64)
    tile_rc = np.empty(n_tiles, np.int64)
    t = 0
    for ch in range(Q):
        full, rem = divmod(C_ch[ch], TILE_COLS)
        for k2 in range(C_ch_pad[ch] // TILE_COLS):
            tile_ch[t] = ch
            tile_rc[t] = TILE_COLS if k2 < full else rem
            t += 1
    assert t == n_tiles

    # idx arrays: wrapped 16-partition layout per tile (512 int16 per
    # partition), replicated across the 8 Q7 cores
    idx_all = np.empty((N_CORES, 128, C_tot * 8), dtype=np.int16)
    indg_all = np.zeros((N_CORES, P, G, 1), dtype=np.float32)
    for i in range(N_CORES):
        e_src, e_dstl, e_ch, rank_to_node = per_core[i]
        rank_of = np.full(NPC, -1, np.int64)
        real = rank_to_node >= 0
        rank_of[rank_to_node[real]] = np.nonzero(real)[0]
        r_e = rank_of[e_dstl]
        key = r_e * Q + e_ch
        perm = np.argsort(key, kind="stable")
        ks = key[perm]
        starts = np.r_[0, np.nonzero(np.diff(ks))[0] + 1]
        runlen = np.diff(np.r_[starts, len(ks)])
        j_sorted = np.arange(len(ks)) - np.repeat(starts, runlen)
        j_e = np.empty(len(ks), np.int64)
        j_e[perm] = j_sorted
        col_l = np.empty(len(ks), np.int64)
        for ch in range(Q):
            mm = e_ch == ch
            col_l[mm] = colpos[ch][j_e[mm], r_e[mm] // P]
        assert (col_l >= 0).all()
        slots = np.full(C_tot * 128, R - 1, dtype=np.int16)  # pad -> zero row
        lin = (chunk_col_base[e_ch] + col_l) * 128 + (r_e % P)
        slots[lin] = cpos[e_src].astype(np.int16)
        sl = slots.reshape(-1, 8192)
        w = sl.reshape(-1, 512, 16).transpose(0, 2, 1)      # [tiles, 16, 512]
        w16 = np.ascontiguousarray(w.transpose(1, 0, 2)).reshape(16, -1)
        wr = idx_all[i]
        for r2 in range(8):
            wr[16 * r2:16 * (r2 + 1)] = w16
        rr = np.arange(RANKS)
        indg_all[i, rr[real] % P, rr[real] // P, 0] = in_dg[
            i * NPC + rank_to_node[real], 0
        ]

    # matmul segments
    segs = []  # (tile, col_start_in_tile, width, g_start, class, j)
    prev = None
    for posn, (ch, j, g) in enumerate(cols):
        if j < 0:
            if prev is not None:
                segs.append(tuple(prev))
                prev = None
            continue
        t, cs = divmod(posn, TILE_COLS)
        if (
            prev is not None
            and prev[4] == ch
            and prev[5] == j
            and g == prev[3] + prev[2]
            and t == prev[0]
            and g // BANK == prev[3] // BANK
        ):
            prev[2] += 1
        else:
            if prev is not None:
                segs.append(tuple(prev))
            prev = [t, cs, 1, g, ch, j]
    if prev is not None:
        segs.append(tuple(prev))

    # class-permuted, padded node tables (pure layout: copies only)
    nf_perm = np.zeros((Q * R, D), dtype=np.float32)
    od_perm = np.zeros((Q * R, 1), dtype=np.float32)
    nodes = np.arange(N)
    pos_glob = color.astype(np.int64) * R + cpos
    nf_perm[pos_glob] = node_f
    od_perm[pos_glob] = out_d

    return dict(
        nf=nf_perm,
        od=od_perm,
        idx=idx_all,
        indg=indg_all,
        rank_to_node=[pc[3] for pc in per_core],
        segs=segs,
        C_tot=C_tot,
        tile_ch=tile_ch.tolist(),
        tile_rc=tile_rc.tolist(),
        slots_real=int(len(src)),
        slots_total=int(C_tot * 128 * N_CORES),
    )


# ---------------------------------------------------------------- device


def _build_nc(C_tot, tile_ch, tile_rc, segs):
    import concourse.bass as bass
    import concourse.tile as tile
    from concourse import bacc, mybir

    nc = bacc.Bacc(
        "TRN2", target_bir_lowering=False, debug=False,
        dynamic_dma_scratch_size=32768,
    )
    nf_d = nc.dram_tensor(
        "nf", [Q * R, D], mybir.dt.float32, kind="ExternalInput"
    ).ap()
    od_d = nc.dram_tensor(
        "od", [Q * R, 1], mybir.dt.float32, kind="ExternalInput"
    ).ap()
    idx_d = nc.dram_tensor(
        "idx", [128, C_tot * 8], mybir.dt.int16, kind="ExternalInput"
    ).ap()
    indg_d = nc.dram_tensor(
        "indg", [P, G, 1], mybir.dt.float32, kind="ExternalInput"
    ).ap()
    out_dram = nc.dram_tensor(
        "out", [P, G, D], mybir.dt.float32, kind="ExternalOutput"
    ).ap()

    def raw_dma_gather(out_ap, in_ap, idxs_ap, num_idxs, elem_size, elem_step):
        eng = nc.gpsimd
        stride_bytes = elem_step * mybir.dt.size(in_ap.dtype)
        _in_ap = eng.lower_ap_dma(in_ap, for_custom_bir_dma=True)
        return eng.add_instruction(
            mybir.InstDMAGatherAnt(
                name=eng.bass.get_next_instruction_name(),
                ins=[*_in_ap, eng.lower_ap(idxs_ap),
                     eng.lower_val_access(eng.to_reg(num_idxs))],
                outs=[eng.lower_ap(out_ap)],
                transpose=False, num_idxs=num_idxs, elem_size=elem_size,
                stride_bytes_256=stride_bytes // 256, gen_mode=0,
                single_packet=False, queue_num=0, sbuf_tokens_per_rank=0,
                sbuf_free_dim_per_rank=0, sbuf_free_dim_pad_per_rank=0,
                sbuf_byte_offset=0,
            )
        )

    last_of_bank = {}
    for si, s in enumerate(segs):
        last_of_bank[s[3] // BANK] = si
    segs_by_tile = {}
    for si, s in enumerate(segs):
        segs_by_tile.setdefault(s[0], []).append((si, s))
    n_tiles = C_tot // TILE_COLS
    # tiles of each class, in order
    tiles_of = {ch: [t for t in range(n_tiles) if tile_ch[t] == ch]
                for ch in range(Q)}

    with tile.TileContext(nc) as tc:
        with (
            tc.tile_pool(name="persist", bufs=1) as persist,
            tc.tile_pool(name="build", bufs=1) as build_pool,
            tc.tile_pool(name="tabp", bufs=1, space="DRAM") as tab_pool,
            tc.tile_pool(name="msgs", bufs=4) as msgs_pool,
            tc.psum_pool(name="psum", bufs=1) as psum_pool,
        ):
            # early loads
            od_sb = build_pool.tile([128, R // 32, 1], mybir.dt.bfloat16)
            nc.gpsimd.dma_start(out=od_sb[:], in_=od_d[:])
            idx_t = persist.tile([128, C_tot * 8], mybir.dt.int16)
            nc.sync.dma_start(out=idx_t[:], in_=idx_d[:])
            indg_t = persist.tile([P, G, 1], mybir.dt.float32)
            nc.sync.dma_start(out=indg_t[:], in_=indg_d[:])

            h_sb = build_pool.tile([128, R // 32, D], mybir.dt.bfloat16)
            nc.gpsimd.dma_start(out=h_sb[:], in_=nf_d[:])
            nc.vector.tensor_tensor(
                out=h_sb[:], in0=h_sb[:],
                in1=od_sb[:].to_broadcast([128, R // 32, D]),
                op=mybir.AluOpType.mult,
            )

            it = persist.tile([P, P], mybir.dt.int32)
            nc.gpsimd.iota(it[:], pattern=[[1, P]], base=0, channel_multiplier=-1)
            ident = persist.tile([P, P], mybir.dt.bfloat16)
            nc.vector.tensor_scalar(
                out=ident[:], in0=it[:], scalar1=0, scalar2=None,
                op0=mybir.AluOpType.is_equal,
            )

            pt = psum_pool.tile([P, NB * BANK, D], mybir.dt.float32)
            res = persist.tile([P, G, D], mybir.dt.float32)

            tables = [
                tab_pool.tile([R, TW], mybir.dt.bfloat16, name=f"tab{q}")
                for q in range(Q)
            ]

            for ch in range(Q):
                nc.sync.dma_start(
                    out=tables[ch][0:R, 0:D],
                    in_=h_sb[32 * ch:32 * (ch + 1), :, :],
                )
                for t in tiles_of[ch]:
                    rc = tile_rc[t]
                    if rc == 0:
                        continue
                    m = msgs_pool.tile([P, TILE_COLS, D], mybir.dt.bfloat16,
                                       tag="m")
                    raw_dma_gather(
                        m[:, 0:rc, :],
                        tables[ch][0:R, 0:D],
                        idx_t[:, t * 512:t * 512 + rc * 8],
                        rc * 128, D, TW,
                    )
                    for si, (tt, cs, w, gs, sch, j) in segs_by_tile.get(t, []):
                        nc.tensor.matmul(
                            pt[:, gs:gs + w, :],
                            ident[:],
                            m[:, cs:cs + w, :],
                            start=(sch == 0 and j == 0),
                            stop=(last_of_bank[gs // BANK] == si),
                        )
                        b = gs // BANK
                        if last_of_bank[b] == si:
                            g0 = b * BANK
                            wb = min(BANK, G - g0)
                            nc.vector.tensor_tensor(
                                out=res[:, g0:g0 + wb, :],
                                in0=pt[:, g0:g0 + wb, :],
                                in1=indg_t[:, g0:g0 + wb, :].to_broadcast(
                                    [P, wb, D]),
                                op=mybir.AluOpType.mult,
                            )
                            nc.sync.dma_start(
                                out=out_dram[:, g0:g0 + wb, :],
                                in_=res[:, g0:g0 + wb, :],
                            )
    nc.compile()
    return nc


# ---------------------------------------------------------------- entry


last_run_info = {}


def kernel(node_f, out_d, in_dg, src, dst, *, _trace=False):
    node_f = np.asarray(node_f, dtype=np.float32)
    out_d = np.asarray(out_d, dtype=np.float32)
    in_dg = np.asarray(in_dg, dtype=np.float32)
    src = np.asarray(src)
    dst = np.asarray(dst)

    pp = _preprocess(node_f, out_d, in_dg, src, dst)

    key = (pp["C_tot"], len(pp["segs"]), tuple(pp["tile_rc"]))
    if key not in _cache:
        _cache.clear()
        _cache[key] = _build_nc(
            pp["C_tot"], pp["tile_ch"], pp["tile_rc"], pp["segs"]
        )
    nc = _cache[key]

    from concourse.bass_utils import run_bass_kernel_spmd

    in_maps = [
        {"nf": pp["nf"], "od": pp["od"], "idx": pp["idx"][i], "indg": pp["indg"][i]}
        for i in range(N_CORES)
    ]
    # Sacrificial device touch: after an earlier crashed session the first
    # device interaction can report NRT_EXEC_UNIT_UNRECOVERABLE once and
    # then recover; absorb that here instead of failing the real run.
    try:
        import jax
        import jax.numpy as jnp

        jnp.zeros((2,)).block_until_ready()
    except Exception:
        pass

    trace_kwargs = (
        dict(trace=True, trace_cores=list(range(N_CORES))) if _trace else {}
    )
    res = None
    for attempt in range(3):
        try:
            res = run_bass_kernel_spmd(
                nc, in_maps, core_ids=list(range(N_CORES)), **trace_kwargs
            )
            break
        except ModuleNotFoundError:
            trace_kwargs = {}
        except Exception:  # noqa: BLE001
            import time as _time

            _time.sleep(2.0)
    if res is None:
        res = run_bass_kernel_spmd(nc, in_maps, core_ids=list(range(N_CORES)))
    last_run_info["exec_time_ns"] = res.exec_time_ns
    last_run_info["mean_exec_time_ns"] = res.mean_exec_time_ns
    last_run_info["trace"] = res.instructions_and_trace
    last_run_info["pp_stats"] = {
        k: pp[k] for k in ("C_tot", "slots_real", "slots_total")
    }

    out = np.empty((N, D), dtype=np.float32)
    rr = np.arange(RANKS)
    for i in range(N_CORES):
        o = res.results[i]["out"]  # [P, G, D]
        r2n = pp["rank_to_node"][i]
        real = r2n >= 0
        out[i * NPC + r2n[real]] = o[rr[real] % P, rr[real] // P]
    return out


# revision 7
# speedup vs baseline: 7.3676x; 1.0555x over previous
"""GCN layer (gather + segment_sum + scale) on 8 Trainium2 NeuronCores.

Strategy (1D destination-node parallel, dma_gather based):
  - Host (integer/index/layout work only): shard edges by dst block of 12500
    nodes. Sources are 4-colored (greedy, balancing each destination's
    in-edges across colors) because dma_gather indices are int16: each color
    class gets its own table of <= 25088 rows. Per core, nodes are packed
    into 98 groups of 128 ranks by a greedy that minimizes per-color degree
    maxima. Slot (p, c): column c belongs to (class, layer j, group g);
    row p = node rank % 128. Padding slots point at a zeroed pad row.
  - Device (all FP math): build prescaled bf16 class tables h = node_f*out_d
    in DRAM with 256B row stride (dma_gather ISA requirement), then one
    dma_gather per <=64 columns (<=8192 indices, 64B payloads). The Tensor
    engine accumulates slot columns into PSUM (fp32) via identity matmuls;
    finalize multiplies by in_dg on DVE and DMAs out.
  - Host: inverse-permute rows back to original node order.
"""

import sys

import numpy as np

for _p in ("/opt/trn_rl_repo", "/root/.axon_site/_ro/trn_rl_repo"):
    if _p not in sys.path:
        sys.path.append(_p)

P = 128
D = 32
N_CORES = 8
N = 100000
NPC = N // N_CORES            # 12500 dst nodes per core
G = (NPC + P - 1) // P        # 98 groups
RANKS = G * P                 # 12544
Q = 4                         # color classes (int16 addressing limit)
R = 25088                     # class table rows (= 32*784)
CLASS_CAP = R - 64            # max real nodes per class
TW = 128                      # table row width in bf16 elems (256B stride)
TILE_COLS = 64                # max columns per dma_gather (8192 idxs)
BANK = 16                     # psum groups per bank
NB = (G + BANK - 1) // BANK   # 7 banks

_cache = {}


# ---------------------------------------------------------------- host prep


def _color_sources(src, dst):
    """4-color source nodes, balancing each dst's in-edges across colors.

    Returns color [N] int8 and pos [N] (position within class).
    """
    deg_tot = np.bincount(dst, minlength=N)
    cap = np.ceil(deg_tot / Q).astype(np.int32)
    es = np.argsort(src, kind="stable")
    d_sorted = dst[es]
    starts = np.searchsorted(src[es], np.arange(N + 1))
    out_deg = np.diff(starts)
    order = np.argsort(-out_deg, kind="stable")
    deg_q = np.zeros((N, Q), np.int32)
    class_sz = np.zeros(Q, np.int64)
    color = np.empty(N, np.int8)
    for u in order:
        vs = d_sorted[starts[u]:starts[u + 1]]
        if len(vs) == 0:
            q = int(class_sz.argmin())
        else:
            dq = deg_q[vs]
            over = np.maximum(dq + 1 - cap[vs][:, None], 0).sum(axis=0)
            over = over.astype(np.int64) * (1 << 20) + class_sz
            over[class_sz >= CLASS_CAP] = 1 << 60
            q = int(over.argmin())
            deg_q[vs, q] += 1
        color[u] = q
        class_sz[q] += 1
    pos = np.empty(N, np.int64)
    for q in range(Q):
        mq = np.nonzero(color == q)[0]
        pos[mq] = np.arange(len(mq))
    return color, pos, class_sz


def _group_nodes(deg_ch):
    """Greedy pack nodes into groups of P minimizing sum of per-class maxima."""
    order = np.argsort(-deg_ch.max(axis=1), kind="stable")
    gmax = np.zeros((G, Q), np.int64)
    gsum = np.zeros(G, np.int64)
    gcnt = np.zeros(G, np.int64)
    assign = np.empty(NPC, np.int64)
    BIG = 1 << 40
    for v in order:
        d = deg_ch[v]
        inc = np.maximum(gmax, d).sum(axis=1) - gsum
        inc[gcnt >= P] = BIG
        g = int(inc.argmin())
        assign[v] = g
        np.maximum(gmax[g], d, out=gmax[g])
        gsum[g] = gmax[g].sum()
        gcnt[g] += 1
    rank_to_node = np.full(RANKS, -1, np.int64)
    posg = np.zeros(G, np.int64)
    for v in range(NPC):
        g = assign[v]
        rank_to_node[g * P + posg[g]] = v
        posg[g] += 1
    return rank_to_node, gmax


def _preprocess(node_f, out_d, in_dg, src, dst):
    src = src.astype(np.int64)
    dst = dst.astype(np.int64)
    color, cpos, class_sz = _color_sources(src, dst)
    e_col = color[src].astype(np.int64)

    core_of = dst // NPC
    per_core = []
    k_sorted_all = []
    for i in range(N_CORES):
        m = core_of == i
        e_src = src[m]
        e_dstl = dst[m] - i * NPC
        e_ch = e_col[m]
        deg_ch = np.zeros((NPC, Q), np.int64)
        for c in range(Q):
            deg_ch[:, c] = np.bincount(e_dstl[e_ch == c], minlength=NPC)
        rank_to_node, k = _group_nodes(deg_ch)
        gorder = np.argsort(-k.sum(axis=1), kind="stable")
        k_sorted = k[gorder]
        rank_to_node = rank_to_node.reshape(G, P)[gorder].reshape(RANKS)
        per_core.append((e_src, e_dstl, e_ch, rank_to_node))
        k_sorted_all.append(k_sorted)

    k_shared = np.maximum.reduce(k_sorted_all)      # [G, Q]
    k_shared[:, 0] = np.maximum(k_shared[:, 0], 1)  # class-0 layer-0 full init

    # global column order: class-major, then layer j, then group g ascending
    cols = []          # (q, j, g); (q, -1, -1) for pad
    colpos = {}        # q -> [kmax_q, G] chunk-local col position
    C_ch, C_ch_pad = [], []
    for ch in range(Q):
        kmax = int(k_shared[:, ch].max())
        cp = np.full((max(kmax, 1), G), -1, np.int64)
        c = 0
        for j in range(kmax):
            for g in range(G):
                if k_shared[g, ch] > j:
                    cp[j, g] = c
                    cols.append((ch, j, g))
                    c += 1
        pad = (-c) % TILE_COLS
        colpos[ch] = cp
        C_ch.append(c)
        C_ch_pad.append(c + pad)
        cols.extend([(ch, -1, -1)] * pad)
    C_tot = sum(C_ch_pad)
    chunk_col_base = np.concatenate([[0], np.cumsum(C_ch_pad)])[:Q]

    # per-tile: class + number of real columns
    n_tiles = C_tot // TILE_COLS
    tile_ch = np.empty(n_tiles, np.int64)
    tile_rc = np.empty(n_tiles, np.int64)
    t = 0
    for ch in range(Q):
        full, rem = divmod(C_ch[ch], TILE_COLS)
        for k2 in range(C_ch_pad[ch] // TILE_COLS):
            tile_ch[t] = ch
            tile_rc[t] = TILE_COLS if k2 < full else rem
            t += 1
    assert t == n_tiles

    # idx arrays: wrapped 16-partition layout per tile (512 int16 per
    # partition), replicated across the 8 Q7 cores
    idx_all = np.empty((N_CORES, 128, C_tot * 8), dtype=np.int16)
    indg_all = np.zeros((N_CORES, P, G, 1), dtype=np.float32)
    for i in range(N_CORES):
        e_src, e_dstl, e_ch, rank_to_node = per_core[i]
        rank_of = np.full(NPC, -1, np.int64)
        real = rank_to_node >= 0
        rank_of[rank_to_node[real]] = np.nonzero(real)[0]
        r_e = rank_of[e_dstl]
        key = r_e * Q + e_ch
        perm = np.argsort(key, kind="stable")
        ks = key[perm]
        starts = np.r_[0, np.nonzero(np.diff(ks))[0] + 1]
        runlen = np.diff(np.r_[starts, len(ks)])
        j_sorted = np.arange(len(ks)) - np.repeat(starts, runlen)
        j_e = np.empty(len(ks), np.int64)
        j_e[perm] = j_sorted
        col_l = np.empty(len(ks), np.int64)
        for ch in range(Q):
            mm = e_ch == ch
            col_l[mm] = colpos[ch][j_e[mm], r_e[mm] // P]
        assert (col_l >= 0).all()
        slots = np.full(C_tot * 128, R - 1, dtype=np.int16)  # pad -> zero row
        lin = (chunk_col_base[e_ch] + col_l) * 128 + (r_e % P)
        slots[lin] = cpos[e_src].astype(np.int16)
        sl = slots.reshape(-1, 8192)
        w = sl.reshape(-1, 512, 16).transpose(0, 2, 1)      # [tiles, 16, 512]
        w16 = np.ascontiguousarray(w.transpose(1, 0, 2)).reshape(16, -1)
        wr = idx_all[i]
        for r2 in range(8):
            wr[16 * r2:16 * (r2 + 1)] = w16
        rr = np.arange(RANKS)
        indg_all[i, rr[real] % P, rr[real] // P, 0] = in_dg[
            i * NPC + rank_to_node[real], 0
        ]

    # matmul segments
    segs = []  # (tile, col_start_in_tile, width, g_start, class, j)
    prev = None
    for posn, (ch, j, g) in enumerate(cols):
        if j < 0:
            if prev is not None:
                segs.append(tuple(prev))
                prev = None
            continue
        t, cs = divmod(posn, TILE_COLS)
        if (
            prev is not None
            and prev[4] == ch
            and prev[5] == j
            and g == prev[3] + prev[2]
            and t == prev[0]
            and g // BANK == prev[3] // BANK
        ):
            prev[2] += 1
        else:
            if prev is not None:
                segs.append(tuple(prev))
            prev = [t, cs, 1, g, ch, j]
    if prev is not None:
        segs.append(tuple(prev))

    # class-permuted, padded node tables (pure layout: copies only)
    nf_perm = np.zeros((Q * R, D), dtype=np.float32)
    od_perm = np.zeros((Q * R, 1), dtype=np.float32)
    nodes = np.arange(N)
    pos_glob = color.astype(np.int64) * R + cpos
    nf_perm[pos_glob] = node_f
    od_perm[pos_glob] = out_d

    return dict(
        nf=nf_perm,
        od=od_perm,
        idx=idx_all,
        indg=indg_all,
        rank_to_node=[pc[3] for pc in per_core],
        segs=segs,
        C_tot=C_tot,
        tile_ch=tile_ch.tolist(),
        tile_rc=tile_rc.tolist(),
        slots_real=int(len(src)),
        slots_total=int(C_tot * 128 * N_CORES),
    )


# ---------------------------------------------------------------- device


def _build_nc(C_tot, tile_ch, tile_rc, segs):
    import concourse.bass as bass
    import concourse.tile as tile
    from concourse import bacc, mybir

    nc = bacc.Bacc(
        "TRN2", target_bir_lowering=False, debug=False,
        dynamic_dma_scratch_size=32768,
    )
    nf_d = nc.dram_tensor(
        "nf", [Q * R, D], mybir.dt.float32, kind="ExternalInput"
    ).ap()
    od_d = nc.dram_tensor(
        "od", [Q * R, 1], mybir.dt.float32, kind="ExternalInput"
    ).ap()
    idx_d = nc.dram_tensor(
        "idx", [128, C_tot * 8], mybir.dt.int16, kind="ExternalInput"
    ).ap()
    indg_d = nc.dram_tensor(
        "indg", [P, G, 1], mybir.dt.float32, kind="ExternalInput"
    ).ap()
    out_dram = nc.dram_tensor(
        "out", [P, G, D], mybir.dt.float32, kind="ExternalOutput"
    ).ap()

    def raw_dma_gather(out_ap, in_ap, idxs_ap, num_idxs, elem_size, elem_step):
        eng = nc.gpsimd
        stride_bytes = elem_step * mybir.dt.size(in_ap.dtype)
        _in_ap = eng.lower_ap_dma(in_ap, for_custom_bir_dma=True)
        return eng.add_instruction(
            mybir.InstDMAGatherAnt(
                name=eng.bass.get_next_instruction_name(),
                ins=[*_in_ap, eng.lower_ap(idxs_ap),
                     eng.lower_val_access(eng.to_reg(num_idxs))],
                outs=[eng.lower_ap(out_ap)],
                transpose=False, num_idxs=num_idxs, elem_size=elem_size,
                stride_bytes_256=stride_bytes // 256, gen_mode=0,
                single_packet=False, queue_num=0, sbuf_tokens_per_rank=0,
                sbuf_free_dim_per_rank=0, sbuf_free_dim_pad_per_rank=0,
                sbuf_byte_offset=0,
            )
        )

    last_of_bank = {}
    for si, s in enumerate(segs):
        last_of_bank[s[3] // BANK] = si
    segs_by_tile = {}
    for si, s in enumerate(segs):
        segs_by_tile.setdefault(s[0], []).append((si, s))
    n_tiles = C_tot // TILE_COLS
    # tiles of each class, in order
    tiles_of = {ch: [t for t in range(n_tiles) if tile_ch[t] == ch]
                for ch in range(Q)}

    with tile.TileContext(nc) as tc:
        with (
            tc.tile_pool(name="persist", bufs=1) as persist,
            tc.tile_pool(name="build", bufs=1) as build_pool,
            tc.tile_pool(name="tabp", bufs=1, space="DRAM") as tab_pool,
            tc.tile_pool(name="msgs", bufs=4) as msgs_pool,
            tc.psum_pool(name="psum", bufs=1) as psum_pool,
        ):
            # early loads; od is expanded to a packed bf16 tile on DVE while
            # the nf and idx DMAs stream in, so the h multiply runs in 2x
            # mode and stalls the table writes as little as possible
            od_sb = build_pool.tile([128, R // 32, 1], mybir.dt.bfloat16)
            nc.gpsimd.dma_start(out=od_sb[:], in_=od_d[:])
            h_sb = build_pool.tile([128, R // 32, D], mybir.dt.bfloat16)
            nc.gpsimd.dma_start(out=h_sb[:], in_=nf_d[:])
            odx = build_pool.tile([128, R // 32, D], mybir.dt.bfloat16)
            nc.vector.tensor_copy(
                out=odx[:], in_=od_sb[:].to_broadcast([128, R // 32, D])
            )
            idx_t = persist.tile([128, C_tot * 8], mybir.dt.int16)
            nc.sync.dma_start(out=idx_t[:], in_=idx_d[:])
            indg_t = persist.tile([P, G, 1], mybir.dt.float32)
            nc.sync.dma_start(out=indg_t[:], in_=indg_d[:])
            nc.vector.tensor_tensor(
                out=h_sb[:], in0=h_sb[:], in1=odx[:],
                op=mybir.AluOpType.mult,
            )

            it = persist.tile([P, P], mybir.dt.int32)
            nc.gpsimd.iota(it[:], pattern=[[1, P]], base=0, channel_multiplier=-1)
            ident = persist.tile([P, P], mybir.dt.bfloat16)
            nc.vector.tensor_scalar(
                out=ident[:], in0=it[:], scalar1=0, scalar2=None,
                op0=mybir.AluOpType.is_equal,
            )

            pt = psum_pool.tile([P, NB * BANK, D], mybir.dt.float32)
            res = persist.tile([P, G, D], mybir.dt.float32)

            tables = [
                tab_pool.tile([R, TW], mybir.dt.bfloat16, name=f"tab{q}")
                for q in range(Q)
            ]

            for ch in range(Q):
                nc.sync.dma_start(
                    out=tables[ch][0:R, 0:D],
                    in_=h_sb[32 * ch:32 * (ch + 1), :, :],
                )
                for t in tiles_of[ch]:
                    rc = tile_rc[t]
                    if rc == 0:
                        continue
                    m = msgs_pool.tile([P, TILE_COLS, D], mybir.dt.bfloat16,
                                       tag="m")
                    raw_dma_gather(
                        m[:, 0:rc, :],
                        tables[ch][0:R, 0:D],
                        idx_t[:, t * 512:t * 512 + rc * 8],
                        rc * 128, D, TW,
                    )
                    for si, (tt, cs, w, gs, sch, j) in segs_by_tile.get(t, []):
                        nc.tensor.matmul(
                            pt[:, gs:gs + w, :],
                            ident[:],
                            m[:, cs:cs + w, :],
                            start=(sch == 0 and j == 0),
                            stop=(last_of_bank[gs // BANK] == si),
                        )
                        b = gs // BANK
                        if last_of_bank[b] == si:
                            g0 = b * BANK
                            wb = min(BANK, G - g0)
                            nc.vector.tensor_tensor(
                                out=res[:, g0:g0 + wb, :],
                                in0=pt[:, g0:g0 + wb, :],
                                in1=indg_t[:, g0:g0 + wb, :].to_broadcast(
                                    [P, wb, D]),
                                op=mybir.AluOpType.mult,
                            )
                            nc.sync.dma_start(
                                out=out_dram[:, g0:g0 + wb, :],
                                in_=res[:, g0:g0 + wb, :],
                            )
    nc.compile()
    return nc


# ---------------------------------------------------------------- entry


last_run_info = {}


def kernel(node_f, out_d, in_dg, src, dst, *, _trace=False):
    node_f = np.asarray(node_f, dtype=np.float32)
    out_d = np.asarray(out_d, dtype=np.float32)
    in_dg = np.asarray(in_dg, dtype=np.float32)
    src = np.asarray(src)
    dst = np.asarray(dst)

    pp = _preprocess(node_f, out_d, in_dg, src, dst)

    key = (pp["C_tot"], len(pp["segs"]), tuple(pp["tile_rc"]))
    if key not in _cache:
        _cache.clear()
        _cache[key] = _build_nc(
            pp["C_tot"], pp["tile_ch"], pp["tile_rc"], pp["segs"]
        )
    nc = _cache[key]

    from concourse.bass_utils import run_bass_kernel_spmd

    in_maps = [
        {"nf": pp["nf"], "od": pp["od"], "idx": pp["idx"][i], "indg": pp["indg"][i]}
        for i in range(N_CORES)
    ]
    # Sacrificial device touch: after an earlier crashed session the first
    # device interaction can report NRT_EXEC_UNIT_UNRECOVERABLE once and
    # then recover; absorb that here instead of failing the real run.
    try:
        import jax
        import jax.numpy as jnp

        jnp.zeros((2,)).block_until_ready()
    except Exception:
        pass

    trace_kwargs = (
        dict(trace=True, trace_cores=list(range(N_CORES))) if _trace else {}
    )
    res = None
    for attempt in range(3):
        try:
            res = run_bass_kernel_spmd(
                nc, in_maps, core_ids=list(range(N_CORES)), **trace_kwargs
            )
            break
        except ModuleNotFoundError:
            trace_kwargs = {}
        except Exception:  # noqa: BLE001
            import time as _time

            _time.sleep(2.0)
    if res is None:
        res = run_bass_kernel_spmd(nc, in_maps, core_ids=list(range(N_CORES)))
    last_run_info["exec_time_ns"] = res.exec_time_ns
    last_run_info["mean_exec_time_ns"] = res.mean_exec_time_ns
    last_run_info["trace"] = res.instructions_and_trace
    last_run_info["pp_stats"] = {
        k: pp[k] for k in ("C_tot", "slots_real", "slots_total")
    }

    out = np.empty((N, D), dtype=np.float32)
    rr = np.arange(RANKS)
    for i in range(N_CORES):
        o = res.results[i]["out"]  # [P, G, D]
        r2n = pp["rank_to_node"][i]
        real = r2n >= 0
        out[i * NPC + r2n[real]] = o[rr[real] % P, rr[real] // P]
    return out


# revision 9
# speedup vs baseline: 7.7595x; 1.0532x over previous
"""GCN layer (gather + segment_sum + scale) on 8 Trainium2 NeuronCores.

Strategy (1D destination-node parallel, dma_gather based):
  - Host (integer/index/layout work only): shard edges by dst block of 12500
    nodes. Sources are 4-colored (greedy, balancing each destination's
    in-edges across colors) because dma_gather indices are int16: each color
    class gets its own table of <= 25088 rows. Per core, nodes are packed
    into 98 groups of 128 ranks by a greedy that minimizes per-color degree
    maxima. Slot (p, c): column c belongs to (class, layer j, group g);
    row p = node rank % 128. Padding slots point at a zeroed pad row.
  - Device (all FP math): build prescaled bf16 class tables h = node_f*out_d
    in DRAM with 256B row stride (dma_gather ISA requirement), then one
    dma_gather per <=64 columns (<=8192 indices, 64B payloads). The Tensor
    engine accumulates slot columns into PSUM (fp32) via identity matmuls;
    finalize multiplies by in_dg on DVE and DMAs out.
  - Host: inverse-permute rows back to original node order.
"""

import sys

import numpy as np

for _p in ("/opt/trn_rl_repo", "/root/.axon_site/_ro/trn_rl_repo"):
    if _p not in sys.path:
        sys.path.append(_p)

P = 128
D = 32
N_CORES = 8
N = 100000
NPC = N // N_CORES            # 12500 dst nodes per core
G = (NPC + P - 1) // P        # 98 groups
RANKS = G * P                 # 12544
Q = 4                         # color classes (int16 addressing limit)
XPP = 783                     # table build: free-dim length per partition
CLASS_PARTS = [8, 40, 40, 40]  # build partitions per class (sum 128)
R_Q = [p * XPP for p in CLASS_PARTS]   # class table rows [6264, 31320, ...]
PART_OFF = [0, 8, 48, 88]
CLASS_CAP = [r - 64 for r in R_Q]      # max real nodes per class
TW = 128                      # table row width in bf16 elems (256B stride)
TILE_COLS = 112               # max columns per dma_gather (14336 idxs)
BANK = 16                     # psum groups per bank
NB = (G + BANK - 1) // BANK   # 7 banks

_cache = {}


# ---------------------------------------------------------------- host prep


def _color_sources(src, dst):
    """4-color source nodes, balancing each dst's in-edges across colors.

    Returns color [N] int8 and pos [N] (position within class).
    """
    deg_tot = np.bincount(dst, minlength=N)
    frac = np.array(R_Q, np.float64) / sum(R_Q)
    cap = np.ceil(deg_tot[:, None] * frac[None, :]).astype(np.int32)  # [N, Q]
    es = np.argsort(src, kind="stable")
    d_sorted = dst[es]
    starts = np.searchsorted(src[es], np.arange(N + 1))
    out_deg = np.diff(starts)
    order = np.argsort(-out_deg, kind="stable")
    deg_q = np.zeros((N, Q), np.int32)
    class_sz = np.zeros(Q, np.int64)
    color = np.empty(N, np.int8)
    for u in order:
        vs = d_sorted[starts[u]:starts[u + 1]]
        full = class_sz >= np.array(CLASS_CAP)
        if len(vs) == 0:
            load = class_sz / np.array(R_Q, np.float64)
            load[full] = 2.0
            q = int(load.argmin())
        else:
            dq = deg_q[vs]
            over = np.maximum(dq + 1 - cap[vs], 0).sum(axis=0)
            over = over.astype(np.int64) * (1 << 20) + class_sz
            over[full] = 1 << 60
            q = int(over.argmin())
            deg_q[vs, q] += 1
        color[u] = q
        class_sz[q] += 1
    pos = np.empty(N, np.int64)
    for q in range(Q):
        mq = np.nonzero(color == q)[0]
        pos[mq] = np.arange(len(mq))
    return color, pos, class_sz


def _group_nodes(deg_ch):
    """Greedy pack nodes into groups of P minimizing sum of per-class maxima."""
    order = np.argsort(-deg_ch.max(axis=1), kind="stable")
    gmax = np.zeros((G, Q), np.int64)
    gsum = np.zeros(G, np.int64)
    gcnt = np.zeros(G, np.int64)
    assign = np.empty(NPC, np.int64)
    BIG = 1 << 40
    for v in order:
        d = deg_ch[v]
        inc = np.maximum(gmax, d).sum(axis=1) - gsum
        inc[gcnt >= P] = BIG
        g = int(inc.argmin())
        assign[v] = g
        np.maximum(gmax[g], d, out=gmax[g])
        gsum[g] = gmax[g].sum()
        gcnt[g] += 1
    rank_to_node = np.full(RANKS, -1, np.int64)
    posg = np.zeros(G, np.int64)
    for v in range(NPC):
        g = assign[v]
        rank_to_node[g * P + posg[g]] = v
        posg[g] += 1
    return rank_to_node, gmax


def _preprocess(node_f, out_d, in_dg, src, dst):
    src = src.astype(np.int64)
    dst = dst.astype(np.int64)
    color, cpos, class_sz = _color_sources(src, dst)
    e_col = color[src].astype(np.int64)

    core_of = dst // NPC
    per_core = []
    k_sorted_all = []
    for i in range(N_CORES):
        m = core_of == i
        e_src = src[m]
        e_dstl = dst[m] - i * NPC
        e_ch = e_col[m]
        deg_ch = np.zeros((NPC, Q), np.int64)
        for c in range(Q):
            deg_ch[:, c] = np.bincount(e_dstl[e_ch == c], minlength=NPC)
        rank_to_node, k = _group_nodes(deg_ch)
        gorder = np.argsort(-k.sum(axis=1), kind="stable")
        k_sorted = k[gorder]
        rank_to_node = rank_to_node.reshape(G, P)[gorder].reshape(RANKS)
        per_core.append((e_src, e_dstl, e_ch, rank_to_node))
        k_sorted_all.append(k_sorted)

    k_shared = np.maximum.reduce(k_sorted_all)      # [G, Q]
    k_shared[:, 0] = np.maximum(k_shared[:, 0], 1)  # class-0 layer-0 full init

    # global column order: class-major, then layer j, then group g ascending
    cols = []          # (q, j, g); (q, -1, -1) for pad
    colpos = {}        # q -> [kmax_q, G] chunk-local col position
    C_ch, C_ch_pad = [], []
    for ch in range(Q):
        kmax = int(k_shared[:, ch].max())
        cp = np.full((max(kmax, 1), G), -1, np.int64)
        c = 0
        for j in range(kmax):
            for g in range(G):
                if k_shared[g, ch] > j:
                    cp[j, g] = c
                    cols.append((ch, j, g))
                    c += 1
        pad = (-c) % TILE_COLS
        colpos[ch] = cp
        C_ch.append(c)
        C_ch_pad.append(c + pad)
        cols.extend([(ch, -1, -1)] * pad)
    C_tot = sum(C_ch_pad)
    chunk_col_base = np.concatenate([[0], np.cumsum(C_ch_pad)])[:Q]

    # per-tile: class + number of real columns
    n_tiles = C_tot // TILE_COLS
    tile_ch = np.empty(n_tiles, np.int64)
    tile_rc = np.empty(n_tiles, np.int64)
    t = 0
    for ch in range(Q):
        full, rem = divmod(C_ch[ch], TILE_COLS)
        for k2 in range(C_ch_pad[ch] // TILE_COLS):
            tile_ch[t] = ch
            tile_rc[t] = TILE_COLS if k2 < full else rem
            t += 1
    assert t == n_tiles

    # idx arrays: wrapped 16-partition layout per tile (512 int16 per
    # partition), replicated across the 8 Q7 cores
    idx_all = np.empty((N_CORES, 128, C_tot * 8), dtype=np.int16)
    indg_all = np.zeros((N_CORES, P, G, 1), dtype=np.float32)
    for i in range(N_CORES):
        e_src, e_dstl, e_ch, rank_to_node = per_core[i]
        rank_of = np.full(NPC, -1, np.int64)
        real = rank_to_node >= 0
        rank_of[rank_to_node[real]] = np.nonzero(real)[0]
        r_e = rank_of[e_dstl]
        key = r_e * Q + e_ch
        perm = np.argsort(key, kind="stable")
        ks = key[perm]
        starts = np.r_[0, np.nonzero(np.diff(ks))[0] + 1]
        runlen = np.diff(np.r_[starts, len(ks)])
        j_sorted = np.arange(len(ks)) - np.repeat(starts, runlen)
        j_e = np.empty(len(ks), np.int64)
        j_e[perm] = j_sorted
        col_l = np.empty(len(ks), np.int64)
        for ch in range(Q):
            mm = e_ch == ch
            col_l[mm] = colpos[ch][j_e[mm], r_e[mm] // P]
        assert (col_l >= 0).all()
        slots = np.empty(C_tot * 128, dtype=np.int16)
        for ch in range(Q):                     # pad -> class zero row
            b = chunk_col_base[ch]
            slots[b * 128:(b + C_ch_pad[ch]) * 128] = R_Q[ch] - 1
        lin = (chunk_col_base[e_ch] + col_l) * 128 + (r_e % P)
        slots[lin] = cpos[e_src].astype(np.int16)
        spt = TILE_COLS * 128
        sl = slots.reshape(-1, spt)
        w = sl.reshape(-1, TILE_COLS * 8, 16).transpose(0, 2, 1)
        w16 = np.ascontiguousarray(w.transpose(1, 0, 2)).reshape(16, -1)
        wr = idx_all[i]
        for r2 in range(8):
            wr[16 * r2:16 * (r2 + 1)] = w16
        rr = np.arange(RANKS)
        indg_all[i, rr[real] % P, rr[real] // P, 0] = in_dg[
            i * NPC + rank_to_node[real], 0
        ]

    # matmul segments
    segs = []  # (tile, col_start_in_tile, width, g_start, class, j)
    prev = None
    for posn, (ch, j, g) in enumerate(cols):
        if j < 0:
            if prev is not None:
                segs.append(tuple(prev))
                prev = None
            continue
        t, cs = divmod(posn, TILE_COLS)
        if (
            prev is not None
            and prev[4] == ch
            and prev[5] == j
            and g == prev[3] + prev[2]
            and t == prev[0]
            and g // BANK == prev[3] // BANK
        ):
            prev[2] += 1
        else:
            if prev is not None:
                segs.append(tuple(prev))
            prev = [t, cs, 1, g, ch, j]
    if prev is not None:
        segs.append(tuple(prev))

    # class-permuted, padded node tables (pure layout: copies only)
    off_q = np.concatenate([[0], np.cumsum(R_Q)])[:Q]
    nf_perm = np.zeros((128 * XPP, D), dtype=np.float32)
    od_perm = np.zeros((128 * XPP, 1), dtype=np.float32)
    pos_glob = off_q[color.astype(np.int64)] + cpos
    nf_perm[pos_glob] = node_f
    od_perm[pos_glob] = out_d

    return dict(
        nf=nf_perm,
        od=od_perm,
        idx=idx_all,
        indg=indg_all,
        rank_to_node=[pc[3] for pc in per_core],
        segs=segs,
        C_tot=C_tot,
        tile_ch=tile_ch.tolist(),
        tile_rc=tile_rc.tolist(),
        slots_real=int(len(src)),
        slots_total=int(C_tot * 128 * N_CORES),
    )


# ---------------------------------------------------------------- device


def _build_nc(C_tot, tile_ch, tile_rc, segs):
    import concourse.bass as bass
    import concourse.tile as tile
    from concourse import bacc, mybir

    nc = bacc.Bacc(
        "TRN2", target_bir_lowering=False, debug=False,
        dynamic_dma_scratch_size=32768,
    )
    nf_d = nc.dram_tensor(
        "nf", [128 * XPP, D], mybir.dt.float32, kind="ExternalInput"
    ).ap()
    od_d = nc.dram_tensor(
        "od", [128 * XPP, 1], mybir.dt.float32, kind="ExternalInput"
    ).ap()
    idx_d = nc.dram_tensor(
        "idx", [128, C_tot * 8], mybir.dt.int16, kind="ExternalInput"
    ).ap()
    indg_d = nc.dram_tensor(
        "indg", [P, G, 1], mybir.dt.float32, kind="ExternalInput"
    ).ap()
    out_dram = nc.dram_tensor(
        "out", [P, G, D], mybir.dt.float32, kind="ExternalOutput"
    ).ap()

    def raw_dma_gather(out_ap, in_ap, idxs_ap, num_idxs, elem_size, elem_step):
        eng = nc.gpsimd
        stride_bytes = elem_step * mybir.dt.size(in_ap.dtype)
        _in_ap = eng.lower_ap_dma(in_ap, for_custom_bir_dma=True)
        return eng.add_instruction(
            mybir.InstDMAGatherAnt(
                name=eng.bass.get_next_instruction_name(),
                ins=[*_in_ap, eng.lower_ap(idxs_ap),
                     eng.lower_val_access(eng.to_reg(num_idxs))],
                outs=[eng.lower_ap(out_ap)],
                transpose=False, num_idxs=num_idxs, elem_size=elem_size,
                stride_bytes_256=stride_bytes // 256, gen_mode=0,
                single_packet=False, queue_num=0, sbuf_tokens_per_rank=0,
                sbuf_free_dim_per_rank=0, sbuf_free_dim_pad_per_rank=0,
                sbuf_byte_offset=0,
            )
        )

    last_of_bank = {}
    for si, s in enumerate(segs):
        last_of_bank[s[3] // BANK] = si
    segs_by_tile = {}
    for si, s in enumerate(segs):
        segs_by_tile.setdefault(s[0], []).append((si, s))
    n_tiles = C_tot // TILE_COLS
    # tiles of each class, in order
    tiles_of = {ch: [t for t in range(n_tiles) if tile_ch[t] == ch]
                for ch in range(Q)}

    with tile.TileContext(nc) as tc:
        with (
            tc.tile_pool(name="persist", bufs=1) as persist,
            tc.tile_pool(name="build", bufs=1) as build_pool,
            tc.tile_pool(name="tabp", bufs=1, space="DRAM") as tab_pool,
            tc.tile_pool(name="msgs", bufs=4) as msgs_pool,
            tc.psum_pool(name="psum", bufs=1) as psum_pool,
        ):
            # early loads; od is expanded to a packed bf16 tile on DVE while
            # the nf and idx DMAs stream in, so the h multiply runs in 2x
            # mode and stalls the table writes as little as possible
            od_sb = build_pool.tile([128, XPP, 1], mybir.dt.bfloat16)
            nc.gpsimd.dma_start(out=od_sb[:], in_=od_d[:])
            h_sb = build_pool.tile([128, XPP, D], mybir.dt.bfloat16)
            nc.gpsimd.dma_start(out=h_sb[:], in_=nf_d[:])
            odx = build_pool.tile([128, XPP, D], mybir.dt.bfloat16)
            nc.vector.tensor_copy(
                out=odx[:], in_=od_sb[:].to_broadcast([128, XPP, D])
            )
            idx_t = persist.tile([128, C_tot * 8], mybir.dt.int16)
            nc.gpsimd.dma_start(out=idx_t[:], in_=idx_d[:])
            indg_t = persist.tile([P, G, 1], mybir.dt.float32)
            nc.sync.dma_start(out=indg_t[:], in_=indg_d[:])
            nc.vector.tensor_tensor(
                out=h_sb[:], in0=h_sb[:], in1=odx[:],
                op=mybir.AluOpType.mult,
            )

            it = persist.tile([P, P], mybir.dt.int32)
            nc.gpsimd.iota(it[:], pattern=[[1, P]], base=0, channel_multiplier=-1)
            ident = persist.tile([P, P], mybir.dt.bfloat16)
            nc.vector.tensor_scalar(
                out=ident[:], in0=it[:], scalar1=0, scalar2=None,
                op0=mybir.AluOpType.is_equal,
            )

            pt = psum_pool.tile([P, NB * BANK, D], mybir.dt.float32)
            res = persist.tile([P, G, D], mybir.dt.float32)

            tables = [
                tab_pool.tile([R_Q[q], TW], mybir.dt.bfloat16, name=f"tab{q}")
                for q in range(Q)
            ]

            def emit_write(q):
                nc.sync.dma_start(
                    out=tables[q][0:R_Q[q], 0:D],
                    in_=h_sb[PART_OFF[q]:PART_OFF[q] + CLASS_PARTS[q], :, :],
                )

            emit_write(0)
            emit_write(1)
            for ch in range(Q):
                if ch + 2 < Q:
                    emit_write(ch + 2)
                for t in tiles_of[ch]:
                    rc = tile_rc[t]
                    if rc == 0:
                        continue
                    m = msgs_pool.tile([P, TILE_COLS, D], mybir.dt.bfloat16,
                                       tag="m")
                    raw_dma_gather(
                        m[:, 0:rc, :],
                        tables[ch][0:R_Q[ch], 0:D],
                        idx_t[:, t * TILE_COLS * 8:t * TILE_COLS * 8 + rc * 8],
                        rc * 128, D, TW,
                    )
                    for si, (tt, cs, w, gs, sch, j) in segs_by_tile.get(t, []):
                        nc.tensor.matmul(
                            pt[:, gs:gs + w, :],
                            ident[:],
                            m[:, cs:cs + w, :],
                            start=(sch == 0 and j == 0),
                            stop=(last_of_bank[gs // BANK] == si),
                        )
                        b = gs // BANK
                        if last_of_bank[b] == si:
                            g0 = b * BANK
                            wb = min(BANK, G - g0)
                            nc.vector.tensor_tensor(
                                out=res[:, g0:g0 + wb, :],
                                in0=pt[:, g0:g0 + wb, :],
                                in1=indg_t[:, g0:g0 + wb, :].to_broadcast(
                                    [P, wb, D]),
                                op=mybir.AluOpType.mult,
                            )
                            nc.sync.dma_start(
                                out=out_dram[:, g0:g0 + wb, :],
                                in_=res[:, g0:g0 + wb, :],
                            )
    nc.compile()
    return nc


# ---------------------------------------------------------------- entry


last_run_info = {}


def kernel(node_f, out_d, in_dg, src, dst, *, _trace=False):
    node_f = np.asarray(node_f, dtype=np.float32)
    out_d = np.asarray(out_d, dtype=np.float32)
    in_dg = np.asarray(in_dg, dtype=np.float32)
    src = np.asarray(src)
    dst = np.asarray(dst)

    pp = _preprocess(node_f, out_d, in_dg, src, dst)

    key = (pp["C_tot"], len(pp["segs"]), tuple(pp["tile_rc"]))
    if key not in _cache:
        _cache.clear()
        _cache[key] = _build_nc(
            pp["C_tot"], pp["tile_ch"], pp["tile_rc"], pp["segs"]
        )
    nc = _cache[key]

    from concourse.bass_utils import run_bass_kernel_spmd

    in_maps = [
        {"nf": pp["nf"], "od": pp["od"], "idx": pp["idx"][i], "indg": pp["indg"][i]}
        for i in range(N_CORES)
    ]
    # Sacrificial device touch: after an earlier crashed session the first
    # device interaction can report NRT_EXEC_UNIT_UNRECOVERABLE once and
    # then recover; absorb that here instead of failing the real run.
    try:
        import jax
        import jax.numpy as jnp

        jnp.zeros((2,)).block_until_ready()
    except Exception:
        pass

    trace_kwargs = (
        dict(trace=True, trace_cores=list(range(N_CORES))) if _trace else {}
    )
    res = None
    for attempt in range(3):
        try:
            res = run_bass_kernel_spmd(
                nc, in_maps, core_ids=list(range(N_CORES)), **trace_kwargs
            )
            break
        except ModuleNotFoundError:
            trace_kwargs = {}
        except Exception:  # noqa: BLE001
            import time as _time

            _time.sleep(2.0)
    if res is None:
        res = run_bass_kernel_spmd(nc, in_maps, core_ids=list(range(N_CORES)))
    last_run_info["exec_time_ns"] = res.exec_time_ns
    last_run_info["mean_exec_time_ns"] = res.mean_exec_time_ns
    last_run_info["trace"] = res.instructions_and_trace
    last_run_info["pp_stats"] = {
        k: pp[k] for k in ("C_tot", "slots_real", "slots_total")
    }

    out = np.empty((N, D), dtype=np.float32)
    rr = np.arange(RANKS)
    for i in range(N_CORES):
        o = res.results[i]["out"]  # [P, G, D]
        r2n = pp["rank_to_node"][i]
        real = r2n >= 0
        out[i * NPC + r2n[real]] = o[rr[real] % P, rr[real] // P]
    return out
